# revision 93
# baseline (speedup 1.0000x reference)
"""Trainium2 Bass kernel for nn_DecoderCell (LFADS decoder cell).

Strategy: pure data parallel over 8 NeuronCores (8192 batch rows each),
[feature, batch] on-chip layout, bf16 end-to-end:

- All DRAM I/O, SBUF activations, and matmul operands are bf16 (PSUM fp32).
  Host packs inputs to bf16 and unpacks bf16 outputs; this halves HBM
  traffic and enables DVE 2x/4x fast modes on the gate elementwise ops.
- Matmuls run at N=512 (full super-tile free dim, 1 cycle/row bf16) with
  minimal K-block pass counts; biases ride ones-rows inside packed moving
  blocks so no separate bias ops are needed.
- GRU elementwise: sigmoid synthesized from tanh (one Exp+Tanh table set);
  blend uses z = 0.5*t+0.5 (tensor_scalar, 4x mode) and bf16
  tensor_tensor ops (2x mode) instead of slow scalar_tensor_tensor where
  possible; remaining STT ops are split between DVE and GpSimd.
- 4 DMAs per super-tile (2 in, 2 out) keep the SP queue and HWDGE clear.
"""

import numpy as np
import ml_dtypes

import concourse.bass as bass
import concourse.tile as tile
from concourse import bacc, mybir
from concourse.bass_utils import run_bass_kernel_spmd

BFNP = ml_dtypes.bfloat16

# ---- problem constants (hardcoded; kernel.py must be self-contained) ----
B = 65536
N_CORES = 8
ROWS = B // N_CORES          # 8192 rows per core
NB = 512                     # super-tile batch width (matmul free dim)
NST = ROWS // NB             # 16 super-tiles per core

GEN = 200
CON = 128
CO = 4
LAT = 64
CIE = 128
EXT = 16
CLIP = 5.0
STATE = 420

F32 = mybir.dt.float32
BF16 = mybir.dt.bfloat16
AF = mybir.ActivationFunctionType
ALU = mybir.AluOpType

# weight pack layout: name -> (row0, rows, cols, col_offset)
_WCOLS = {}
_off = 0
for _nm, _r0, _p, _f in (
    ("cwA", 0, 128, 384), ("cwB", 0, 128, 384), ("cwC", 0, 65, 384),
    ("cwH", 0, 128, 384), ("cbH", 0, 65, 128), ("gwA", 0, 128, 600),
    ("gwB", 0, 105, 800), ("coW", 0, 128, 36), ("facW", 0, 128, 128),
):
    _WCOLS[_nm] = (_r0, _p, _f, _off)
    _off += _f
WPACK_COLS = _off

# in1 [128, nst*2560]: per st [ci0 | ci1 | con_s | gen0 | facE] where facE
# col-block rows: fac 0:64 | ones 64 | eps 65:69 | pad.
# misc tile `mt` [117, 512] (genB): gen1 0:72 | ones2 72 | zeros 73:96
# | ext 96:112 | ones 112 | con_out 113:117 (device-written). DMA lands
# rows 0:113. BIR partition rules (<=128 rows from base 0, <=32 from base
# 96) make the matmul blocks: rz/full = mt[0:117] (zero weights on pad
# rows), h-pass = mt[0:73], i-pass = mt[96:117].

# output pack per super-tile, [128, 2048] minus trailing fac pad:
#   0:512     gen0'  (gen gates 0:128)
#   512:1024  con'
#   1024:1536 rows 0:72 gen1' | 72:76 mean | 76:80 std | 80:84 con_out
#   1536:2048 rows 0:64 factor


def build_decoder(nc: bass.Bass, tc: tile.TileContext, ctx, ins, outs,
                  nst: int = NST):
    wp = ctx.enter_context(tc.tile_pool(name="wp", bufs=1))
    lp = ctx.enter_context(tc.tile_pool(name="lp", bufs=6))
    op = ctx.enter_context(tc.tile_pool(name="op", bufs=6))
    gp = ctx.enter_context(tc.tile_pool(name="gp", bufs=4))
    pprz = ctx.enter_context(tc.tile_pool(name="pprz", bufs=2, space="PSUM"))
    pp1 = ctx.enter_context(tc.tile_pool(name="pp1", bufs=3, space="PSUM"))
    ppf = ctx.enter_context(tc.tile_pool(name="ppf", bufs=1, space="PSUM"))

    wsb = wp.tile([128, WPACK_COLS], BF16, name="wsb")
    # con weights land first so con_a(0) can start ~1us earlier; the gen/co
    # halves of the pack arrive in a second DMA.
    _csplit = _WCOLS["gwA"][3]
    nc.sync.dma_start(wsb[:, 0:_csplit], ins["wpack"][:, 0:_csplit])
    nc.sync.dma_start(wsb[:, _csplit:], ins["wpack"][:, _csplit:])
    bv = wp.tile([4, 2], F32, name="bv")
    coBm, coBv = bv[:, 0:1], bv[:, 1:2]
    eps_t = wp.tile([68, ROWS], BF16, name="eps_t")
    io_bv = {"pending": True}

    def load_bv():
        if io_bv.pop("pending", False):
            nc.sync.dma_start(bv[:], ins["biasv"][:])
            nc.sync.dma_start(eps_t[64:68, :], ins["eps"][:])

    def wv(name):
        r0, p, f, c0 = _WCOLS[name]
        return wsb[r0:r0 + p, c0:c0 + f]

    cwA, cwB, cwC, cwH, cbH = wv("cwA"), wv("cwB"), wv("cwC"), wv("cwH"), wv("cbH")
    gwA, gwB, coW, facW = wv("gwA"), wv("gwB"), wv("coW"), wv("facW")

    mm = nc.tensor.matmul

    def stage_load(st):
        c0 = st * 3072
        c1 = slice(st * 512, (st + 1) * 512)
        in1 = lp.tile([128, 3072], BF16, name="in1")   # ci0|ci1|con_s|gen0|facE|gen1
        nc.sync.dma_start(in1[:], ins["in1"][:, c0:c0 + 3072])
        mt = lp.tile([105, 512], BF16, name="mt")      # genB
        nc.sync.dma_start(mt[4:105, :], ins["in2"][:, c1])
        out1 = op.tile([128, 1536], BF16, name="out1")
        out2 = op.tile([68, 512], BF16, name="out2")
        return dict(st=st, in1=in1, mt=mt, out1=out1, out2=out2)

    # ---- one GRU block: matmul phase A (preacts + tanh + tp/u) ----
    # GPSIMD cannot touch PSUM on TRN2, so every PSUM-reading elementwise
    # op (tanh/exp/copies on ACT; tp/u/conout STTs on DVE) stays off Pool;
    # Pool gets SBUF-only bf16 tensor_tensor work (d/m of the blends).
    def gru_a(io, key, prz, pi, ph, sz, u_ap=None, split_t=False):
        """prz/pi/ph already filled by matmuls. Emit tanh + tp/u chain."""
        t = gp.tile([sz, 1024], BF16, name=f"t_{key}", tag=f"t_{key}")
        if split_t:
            nc.scalar.activation(t[:, 0:512], prz[:, 0:512], AF.Tanh,
                                 scale=0.5)
            nc.scalar.activation(t[:, 512:1024], prz[:, 512:1024], AF.Tanh,
                                 scale=0.5)
        else:
            nc.scalar.activation(t[:], prz[:], AF.Tanh, scale=0.5)
        tp = gp.tile([sz, 512], BF16, name=f"tp_{key}", tag=f"tp_{key}")
        nc.vector.scalar_tensor_tensor(  # (1+tanh_r)*h_n  (= 2*r*h_n)
            tp[:], t[:, 0:512], 1.0, ph[:], op0=ALU.add, op1=ALU.mult)
        if u_ap is None:
            u = gp.tile([sz, 512], BF16, name=f"u_{key}", tag=f"u_{key}")
            u_ap = u[:]
            io[f"u_{key}"] = u
        nc.vector.scalar_tensor_tensor(  # 0.5*tp + i_n
            u_ap, tp[:], 0.5, pi[:], op0=ALU.mult, op1=ALU.add)
        io[f"t_{key}"] = t

    # ---- one GRU block: blend phase -> outp slice ----
    # Whole chains stay on one engine (in-order queues hate ping-pong).
    # DVE form exploits 2x/4x fast modes; the Pool form uses STT (0.6 eff)
    # to fold the z affine and halving, since Pool TT runs at 0.42 eff.
    def gru_blend(io, key, n_ap, h_ap, outp, sz, pool=False):
        t = io.pop(f"t_{key}")
        d = gp.tile([sz, 512], BF16, name=f"d_{key}", tag=f"d_{key}")
        if pool:
            # GPSIMD supports only tensor_tensor/tensor_scalar/copy
            nc.gpsimd.tensor_tensor(d[:], h_ap, n_ap, op=ALU.subtract)
            z = gp.tile([sz, 512], BF16, name=f"z_{key}", tag=f"z_{key}")
            nc.gpsimd.tensor_scalar(z[:], t[:, 512:1024], 0.5, 0.5,
                                    op0=ALU.mult, op1=ALU.add)
            m = gp.tile([sz, 512], BF16, name=f"m_{key}", tag=f"m_{key}")
            nc.gpsimd.tensor_tensor(m[:], z[:], d[:], op=ALU.mult)
            c = gp.tile([sz, 512], BF16, name=f"c_{key}", tag=f"c_{key}")
            nc.gpsimd.tensor_tensor(c[:], n_ap, m[:], op=ALU.add)
            nc.gpsimd.tensor_scalar(outp, c[:], CLIP, -CLIP,
                                    op0=ALU.min, op1=ALU.max)
            return
        nc.vector.tensor_tensor(d[:], h_ap, n_ap, op=ALU.subtract)
        z = gp.tile([sz, 512], BF16, name=f"z_{key}", tag=f"z_{key}")
        nc.gpsimd.tensor_scalar(z[:], t[:, 512:1024], 0.5, 0.5,
                                op0=ALU.mult, op1=ALU.add)
        m = gp.tile([sz, 512], BF16, name=f"m_{key}", tag=f"m_{key}")
        nc.vector.tensor_tensor(m[:], z[:], d[:], op=ALU.mult)
        c = gp.tile([sz, 512], BF16, name=f"c_{key}", tag=f"c_{key}")
        nc.vector.tensor_tensor(c[:], n_ap, m[:], op=ALU.add)
        nc.vector.tensor_scalar(outp, c[:], CLIP, -CLIP,
                                op0=ALU.min, op1=ALU.max)

    def stage_con_a(io):
        in1, mt = io["in1"], io["mt"]
        ci0, ci1 = in1[:, 0:512], in1[:, 512:1024]
        con_s = in1[:, 1024:1536]
        facE = in1[0:65, 2048:2560]
        prz = pprz.tile([128, 1024], F32, name="prz_c", tag="rz")
        for g, c0 in ((0, 0), (1, 128)):
            d = prz[:, g * 512:(g + 1) * 512]
            mm(d, cwA[:, c0:c0 + 128], ci0, start=True, stop=False)
            mm(d, cwB[:, c0:c0 + 128], ci1, start=False, stop=False)
            mm(d, cwC[:, c0:c0 + 128], facE, start=False, stop=False)
            mm(d, cwH[:, c0:c0 + 128], con_s, start=False, stop=True)
        pi = pp1.tile([128, 512], F32, name="pi_c", tag="ih")
        mm(pi[:], cwA[:, 256:384], ci0, start=True, stop=False)
        mm(pi[:], cwB[:, 256:384], ci1, start=False, stop=False)
        mm(pi[:], cwC[:, 256:384], facE, start=False, stop=True)
        ph = pp1.tile([128, 512], F32, name="ph_c", tag="ih")
        mm(ph[:], cwH[:, 256:384], con_s, start=True, stop=False)
        mm(ph[:], cbH[:], facE, start=False, stop=True)
        gru_a(io, "c", prz, pi, ph, 128)

    def stage_con_b(io):
        u = io.pop("u_c")
        n = gp.tile([128, 512], BF16, name="n_c", tag="n_c")
        nc.scalar.activation(n[:], u[:], AF.Tanh)
        gru_blend(io, "c", n[:], io["in1"][:, 1024:1536],
                  io["out1"][:, 512:1024], 128)

    def stage_co_mm(io):
        out1 = io["out1"]
        # pco [36, 512]: logvar at rows 0:4 (ACT-readable base 0), mean
        # part at rows 32:36 (DVE-readable base 32).
        pco = ppf.tile([36, 512], F32, name="pco", tag="cf")
        io["pco"] = pco
        mm(pco[:], coW[:], out1[:, 512:1024], start=True, stop=True)
        # std = exp(0.5*logvar_raw + 0.5*b_v) straight into the out2 slot;
        # eps is staged at partitions 64:68 so q satisfies the equal-base
        # rule without a separate std copy.
        nc.scalar.activation(io["out2"][64:68, :], pco[0:4, :], AF.Exp,
                             scale=0.5, bias=coBv)

    def stage_co_fin(io):
        mt, pco = io["mt"], io.pop("pco")
        st = io["st"]
        q = gp.tile([68, 512], BF16, name="q_co")
        nc.vector.tensor_tensor(q[64:68, :], io["out2"][64:68, :],
                                eps_t[64:68, st * 512:(st + 1) * 512],
                                op=ALU.mult)
        # mean = mean_raw + b_m via ACT (drains PSUM), then con_out =
        # mean + std*eps on DVE's fast bf16 path -> genB rows 0:4. The
        # con_out / mean output columns are host-derived.
        mr = gp.tile([68, 512], BF16, name="mr_co")
        nc.scalar.activation(mr[64:68, :], pco[32:36, :], AF.Identity,
                             bias=coBm)
        nc.vector.tensor_tensor(mt[0:4, :], q[64:68, :], mr[64:68, :],
                                op=ALU.add)

    def stage_gen_mm(io):
        in1, mt = io["in1"], io["mt"]
        gen0 = in1[:, 1536:2048]
        genB = mt[0:105, :]
        for key, m0, sz in (("g0", 0, 128), ("g1", 128, 72)):
            prz = pprz.tile([sz, 1024], F32, name=f"prz_{key}", tag="rz")
            io[f"prz_{key}"] = prz
            for g, c0 in ((0, m0), (1, 200 + m0)):
                d = prz[:, g * 512:(g + 1) * 512]
                mm(d, gwA[:, c0:c0 + sz], gen0, start=True, stop=False)
                mm(d, gwB[:, c0:c0 + sz], genB, start=False, stop=True)
        for key, m0, sz in (("g0", 0, 128), ("g1", 128, 72)):
            pi = pp1.tile([sz, 512], F32, name=f"pi_{key}", tag="ih")
            io[f"pi_{key}"] = pi
            mm(pi[:], gwB[0:21, 600 + m0:600 + m0 + sz], mt[0:21, :],
               start=True, stop=True)
            ph = pp1.tile([sz, 512], F32, name=f"ph_{key}", tag="ih")
            io[f"ph_{key}"] = ph
            mm(ph[:], gwA[:, 400 + m0:400 + m0 + sz], gen0,
               start=True, stop=False)
            mm(ph[:], gwB[0:105, 400 + m0:400 + m0 + sz], mt[0:105, :],
               start=False, stop=True)

    def stage_gen_elem(io):
        u_g = gp.tile([128, 1024], BF16, name="u_g", tag="u_g")
        io["u_g"] = u_g
        for key, sz, u_ap in (("g0", 128, u_g[:, 0:512]),
                              ("g1", 72, u_g[0:72, 512:1024])):
            gru_a(io, key, io.pop(f"prz_{key}"), io.pop(f"pi_{key}"),
                  io.pop(f"ph_{key}"), sz, u_ap=u_ap)

    def stage_gen_b(io):
        in1, mt, out1 = io["in1"], io["mt"], io["out1"]
        u_g = io.pop("u_g")
        n_g = gp.tile([128, 1024], BF16, name="n_g", tag="n_g")
        nc.scalar.activation(n_g[:], u_g[:], AF.Tanh)
        gru_blend(io, "g0", n_g[:, 0:512], in1[:, 1536:2048],
                  out1[:, 0:512], 128)
        gru_blend(io, "g1", n_g[0:72, 512:1024], io["in1"][0:72, 2560:3072],
                  out1[0:72, 1024:1536], 72, pool=io["st"] < nst - 2)

    def stage_fac(io):
        out1, out2 = io["out1"], io["out2"]
        pf = ppf.tile([64, 512], F32, name="pf", tag="cf")
        mm(pf[:], facW[:, 0:64], out1[:, 0:512], start=True, stop=False)
        mm(pf[:], facW[0:72, 64:128], out1[0:72, 1024:1536],
           start=False, stop=True)
        nc.scalar.copy(out2[0:64, :], pf[:])

    def stage_store(io):
        st = io["st"]
        nc.sync.dma_start(outs["out1"][:, st * 1536:(st + 1) * 1536],
                          io["out1"][:])
        nc.sync.dma_start(outs["out2"][:, st * 512:(st + 1) * 512],
                          io["out2"][:])

    # 4-stage skewed software pipeline. Per iteration k the PE stream is
    # con_a(k) | fac(k-3) | gen_a(k-2) | co(k): every matmul group has
    # over an iteration of slack between it and the elementwise chain it
    # depends on, so the PE never idles (and never drops out of its fast
    # p-state).
    ios = {}
    ios[0] = stage_load(0)
    load_bv()
    ios[1] = stage_load(1)
    for k in range(nst):
        if k + 2 < nst:
            ios[k + 2] = stage_load(k + 2)
        if k >= 1:
            stage_co_mm(ios[k - 1])
        stage_con_a(ios[k])
        if k >= 4:
            stage_fac(ios[k - 4])
            stage_store(ios[k - 4])
        if k >= 2:
            stage_gen_mm(ios[k - 2])
            stage_gen_elem(ios[k - 2])
        if k >= 1:
            stage_co_fin(ios[k - 1])
        stage_con_b(ios[k])
        if k == nst - 1:
            # last loop pass: pull gen(k-1) forward so the tail only has
            # one super-tile's generator chain left
            stage_gen_mm(ios[k - 1])
        if k >= 2:
            stage_gen_b(ios[k - 2])
        if k == nst - 1:
            stage_gen_elem(ios[k - 1])
    stage_co_mm(ios[nst - 1])
    stage_co_fin(ios[nst - 1])
    stage_fac(ios[nst - 4])
    stage_store(ios[nst - 4])
    stage_gen_b(ios[nst - 2])
    stage_gen_mm(ios[nst - 1])
    stage_gen_elem(ios[nst - 1])
    stage_fac(ios[nst - 3])
    stage_store(ios[nst - 3])
    stage_gen_b(ios[nst - 1])
    for k in (nst - 2, nst - 1):
        stage_fac(ios[k])
        stage_store(ios[k])


def _weight_arrays(gen_w_ih, gen_w_hh, gen_b_ih, gen_b_hh,
                   con_w_ih, con_w_hh, con_b_ih, con_b_hh, co_w, co_b, fac_w):
    f = np.float32
    cw = np.asarray(con_w_ih, f).T                       # [320, 384]
    chh = np.asarray(con_w_hh, f).T                      # [128, 384]
    cbias = np.asarray(con_b_ih, f).copy()
    cbias[:256] += np.asarray(con_b_hh, f)[:256]         # rz merged; n = b_ih
    cwC = np.concatenate([cw[256:320], cbias[None, :]], axis=0)   # [65, 384]
    cbH = np.zeros((65, 128), f)
    cbH[64, :] = np.asarray(con_b_hh, f)[256:384]        # b_hh_n on ones row

    gw = np.asarray(gen_w_ih, f).T                       # [20, 600]
    gh = np.asarray(gen_w_hh, f).T                       # [200, 600]
    gbias = np.asarray(gen_b_ih, f).copy()
    gbias[:400] += np.asarray(gen_b_hh, f)[:400]
    gwB = np.zeros((105, 800), f)
    gwB[0:4, 0:400] = gw[0:4, 0:400]                     # con_out rows (rz)
    gwB[4:20, 0:400] = gw[4:20, 0:400]                   # ext rows (rz)
    gwB[20, 0:400] = gbias[:400]                         # rz bias
    gwB[32:104, 0:400] = gh[128:200, 0:400]              # gen1 rows (rz)
    gwB[32:104, 400:600] = gh[128:200, 400:600]          # gen1 rows (n-h)
    gwB[104, 400:600] = np.asarray(gen_b_hh, f)[400:]    # b_hh_n on ones2
    gwB[0:4, 600:800] = gw[0:4, 400:600]                 # con_out rows (n-i)
    gwB[4:20, 600:800] = gw[4:20, 400:600]               # ext rows (n-i)
    gwB[20, 600:800] = gbias[400:]                       # b_ih_n

    coW36 = np.zeros((128, 36), f)
    coW36[:, 0:4] = np.asarray(co_w, f).T[:, 4:8]        # logvar weights
    coW36[:, 32:36] = np.asarray(co_w, f).T[:, 0:4]      # mean weights

    nrm = np.maximum(np.linalg.norm(np.asarray(fac_w, np.float64), axis=1,
                                    keepdims=True), 1e-12)
    facn = (np.asarray(fac_w, np.float64) / nrm).T.astype(f)      # [200, 64]
    facW = np.zeros((128, 128), f)
    facW[:, 0:64] = facn[0:128]
    facW[0:72, 64:128] = facn[128:200]

    parts = {
        "cwA": cw[0:128], "cwB": cw[128:256], "cwC": cwC, "cwH": chh,
        "cbH": cbH, "gwA": gh[0:128], "gwB": gwB,
        "coW": coW36, "facW": facW,
    }
    wpack = np.zeros((128, WPACK_COLS), dtype=BFNP)
    for nm, (r0, p, fc, c0) in _WCOLS.items():
        wpack[r0:r0 + p, c0:c0 + fc] = parts[nm].astype(BFNP)
    biasv = np.zeros((4, 2), f)
    biasv[:, 0] = np.asarray(co_b, f)[0:4]
    biasv[:, 1] = 0.5 * np.asarray(co_b, f)[4:8]
    return {"wpack": wpack, "biasv": biasv}


_CACHED = {}


def _build_nc(nst=NST):
    if nst in _CACHED:
        return _CACHED[nst]
    from contextlib import ExitStack

    nc = bacc.Bacc("TRN2", target_bir_lowering=False, debug=False,
                   num_devices=N_CORES)
    ins = {
        "in1": nc.dram_tensor("in1", [128, nst * 3072], BF16,
                              kind="ExternalInput").ap(),
        "in2": nc.dram_tensor("in2", [101, nst * 512], BF16,
                              kind="ExternalInput").ap(),
        "eps": nc.dram_tensor("eps", [4, nst * 512], BF16,
                              kind="ExternalInput").ap(),
        "wpack": nc.dram_tensor("wpack", [128, WPACK_COLS], BF16,
                                kind="ExternalInput").ap(),
        "biasv": nc.dram_tensor("biasv", [4, 2], F32,
                                kind="ExternalInput").ap(),
    }
    outs = {
        "out1": nc.dram_tensor("out1", [128, nst * 1536], BF16,
                               kind="ExternalOutput").ap(),
        "out2": nc.dram_tensor("out2", [68, nst * 512], BF16,
                               kind="ExternalOutput").ap(),
    }
    with tile.TileContext(nc) as tc:
        with ExitStack() as ctx:
            build_decoder(nc, tc, ctx, ins, outs, nst=nst)
    nc.compile()
    _CACHED[nst] = nc
    return nc


def pack_inputs(x, h0, eps, rows=ROWS):
    """Host-side bf16 packing of one core's activations."""
    nst = rows // NB
    xT = np.ascontiguousarray(x.T.astype(BFNP))          # [272, rows]
    h0T = np.ascontiguousarray(h0.T.astype(BFNP))        # [420, rows]

    # in1 [128, nst*3072]: per st [ci0 | ci1 | con_s | gen0 | facE | gen1]
    in1 = np.zeros((128, nst, 6, NB), dtype=BFNP)
    in1[:, :, 0, :] = xT[0:128].reshape(128, nst, NB)
    in1[:, :, 1, :] = xT[128:256].reshape(128, nst, NB)
    in1[:, :, 2, :] = h0T[200:328].reshape(128, nst, NB)
    in1[:, :, 3, :] = h0T[0:128].reshape(128, nst, NB)
    in1[0:64, :, 4, :] = h0T[356:420].reshape(64, nst, NB)
    in1[64, :, 4, :] = 1.0
    in1[0:72, :, 5, :] = h0T[128:200].reshape(72, nst, NB)

    # in2 [101, nst*512] -> mt rows 4:105 (genB block)
    genB = np.zeros((101, rows), dtype=BFNP)             # mt rows 4:105
    genB[0:16] = xT[256:272]                             # ext -> rows 4:20
    genB[16] = 1.0                                       # ones -> row 20
    genB[28:100] = h0T[128:200]                          # gen1 -> rows 32:104
    genB[100] = 1.0                                      # ones2 -> row 104
    return {"in1": in1.reshape(128, nst * 3072),
            "in2": np.ascontiguousarray(genB).reshape(101, nst * 512),
            "eps": np.ascontiguousarray(eps.T.astype(BFNP))}


def unpack_outputs(res, x, eps, co_w, co_b, rows=ROWS):
    """Invert the packed output layout into [rows, 420] fp32."""
    nst = rows // NB
    out = np.empty((rows, STATE), dtype=np.float32)
    o1 = np.asarray(res["out1"]).reshape(128, nst, 3, NB)
    out[:, 0:128] = o1[:, :, 0, :].reshape(128, rows).T          # gen0'
    conp = o1[:, :, 1, :].reshape(128, rows).T.astype(np.float32)
    out[:, 200:328] = conp                                       # con'
    out[:, 128:200] = o1[:, :, 2, :][0:72].reshape(72, rows).T   # gen1'
    o2 = np.asarray(res["out2"]).reshape(68, nst, NB)
    out[:, 356:420] = o2[0:64].reshape(64, nst * NB).T           # factor
    std = o2[64:68].reshape(4, nst * NB).T.astype(np.float32)
    out[:, 332:336] = std
    # mean / con_out are host-derived: the device only needs con_out inside
    # the gen input block, which it computes from PSUM directly.
    mean = conp @ np.asarray(co_w, np.float32)[0:4].T + \
        np.asarray(co_b, np.float32)[0:4]
    out[:, 328:332] = mean
    out[:, 336:340] = mean + std * eps
    out[:, 340:356] = x[:, 256:272]                              # ext (exact)
    return out


def kernel(x, h0, eps, gen_w_ih, gen_w_hh, gen_b_ih, gen_b_hh,
           con_w_ih, con_w_hh, con_b_ih, con_b_hh, co_w, co_b, fac_w,
           **run_kwargs):
    x = np.asarray(x, dtype=np.float32)
    h0 = np.asarray(h0, dtype=np.float32)
    eps = np.asarray(eps, dtype=np.float32)
    w = _weight_arrays(gen_w_ih, gen_w_hh, gen_b_ih, gen_b_hh,
                       con_w_ih, con_w_hh, con_b_ih, con_b_hh,
                       co_w, co_b, fac_w)
    nc = _build_nc()

    in_maps = []
    for c in range(N_CORES):
        r0, r1 = c * ROWS, (c + 1) * ROWS
        m = dict(w)
        m.update(pack_inputs(x[r0:r1], h0[r0:r1], eps[r0:r1]))
        in_maps.append(m)

    res = run_bass_kernel_spmd(nc, in_maps, core_ids=list(range(N_CORES)),
                               **run_kwargs)
    out = np.empty((B, STATE), dtype=np.float32)
    for c in range(N_CORES):
        r0, r1 = c * ROWS, (c + 1) * ROWS
        out[r0:r1] = unpack_outputs(res.results[c], x[r0:r1], eps[r0:r1],
                                    co_w, co_b)
    if run_kwargs:
        return out, res
    return out
